# revision 1
# baseline (speedup 1.0000x reference)
"""Trainium2 Bass kernel for nn_EncoderBlock (dense transformer encoder block).

Strategy: pure data parallelism — batch B=8 across the 8 NeuronCores, one
batch element per core. No collectives. Per core:

  LN1 -> q = n@wqT+bq (kh=vh=qh, reproducing the reference's q-reuse bug)
  per head: S = qh^T qh / 8 (symmetric), E = exp(S/8 - 20), Z row-sums via
  activation accum_out (symmetry of S), ctx = E-weighted sum of qh, normalized
  by 1/Z broadcast via DRAM-bounce DMA; wo projection + residual; LN2; ReLU
  FFN (d_ff=4096) streamed from HBM; residual; out.

Matmuls run in bf16 (fp32 accumulation in PSUM); layernorm statistics,
softmax sums and the residual stream stay fp32.
"""

import sys

sys.path.insert(0, "/opt/trn_rl_repo")

import numpy as np
import ml_dtypes
from contextlib import ExitStack

import concourse.bass as bass
import concourse.tile as tile
from concourse import bacc, mybir
from concourse import bass_utils
from concourse.bass import ts, ds
from concourse.masks import make_identity

BF = mybir.dt.bfloat16
F32 = mybir.dt.float32
AF = mybir.ActivationFunctionType
OP = mybir.AluOpType
AX = mybir.AxisListType

P = 128
S = 1024          # sequence length per core
D = 1024          # d_model
H = 16            # heads
DK = 64           # head dim
DFF = 4096
NB = 8            # batch = number of cores
SC = S // P       # 8 sequence chunks
DC = D // P       # 8 feature chunks
FC = DFF // P     # 32 ff chunks
EPS = 1e-6
EXP_SHIFT = -20.0  # constant shift inside exp; cancels in softmax ratio

last_exec_time_ns = None


def _emit_layernorm(nc, small, xt, n_out, alpha, beta, idx, chunks):
    """LN with Bessel-corrected std (ddof=1), matching torch/jax reference:
    n = (x - mu)/(std + eps)*alpha + beta.  xt [P,*,D] f32 indexed by `chunks`,
    n_out [P,len(chunks),D] bf16 indexed locally.
    Stats per token live on partitions ([P, nch] tiles)."""
    chunks = list(chunks)
    nch = len(chunks)
    s1 = small.tile([P, nch], F32, name=f"ln{idx}_s1")
    sq = small.tile([P, nch], F32, name=f"ln{idx}_sq")
    mu = small.tile([P, nch], F32, name=f"ln{idx}_mu")
    var = small.tile([P, nch], F32, name=f"ln{idx}_var")
    tmp = small.tile([P, nch], F32, name=f"ln{idx}_tmp")
    tcoef = small.tile([P, nch], F32, name=f"ln{idx}_t")
    ucoef = small.tile([P, nch], F32, name=f"ln{idx}_u")

    for i, sc in enumerate(chunks):
        nc.vector.reduce_sum(s1[:, ds(i, 1)], xt[:, sc], axis=AX.X)
        # n_out[:, i] doubles as scratch for the squared values (overwritten
        # by the LN apply below).  InstTensorTensorReduce crashes on HW via
        # this toolchain, so the sum of squares comes from ACT Square+accum.
        nc.scalar.activation(
            n_out[:, i], xt[:, sc], AF.Square, accum_out=sq[:, ds(i, 1)],
        )
    nc.vector.tensor_scalar_mul(mu[:], s1[:], 1.0 / D)
    nc.vector.tensor_mul(tmp[:], mu[:], mu[:])
    nc.vector.tensor_scalar_mul(var[:], sq[:], 1.0 / (D - 1))
    nc.vector.tensor_scalar_mul(tmp[:], tmp[:], float(D) / (D - 1))
    nc.vector.tensor_sub(var[:], var[:], tmp[:])
    # std = sqrt(var), ACT sqrt refined with one Newton step:
    # s1 = 0.5*(s0 + var/s0)
    s0 = small.tile([P, nch], F32, name=f"ln{idx}_s0")
    nc.scalar.activation(s0[:], var[:], AF.Sqrt)
    nc.vector.reciprocal(tmp[:], s0[:])
    nc.vector.tensor_mul(tmp[:], tmp[:], var[:])
    nc.vector.tensor_add(tmp[:], tmp[:], s0[:])
    nc.vector.tensor_scalar(tmp[:], tmp[:], 0.5, EPS, OP.mult, OP.add)  # std+eps
    nc.vector.reciprocal(tmp[:], tmp[:])                                # 1/(std+eps)
    nc.vector.tensor_scalar_mul(tcoef[:], tmp[:], float(alpha))
    nc.vector.tensor_mul(tmp[:], mu[:], tcoef[:])
    nc.vector.tensor_scalar(ucoef[:], tmp[:], -1.0, float(beta), OP.mult, OP.add)
    for i, sc in enumerate(chunks):
        nc.vector.tensor_scalar(
            n_out[:, i], xt[:, sc], tcoef[:, ds(i, 1)], ucoef[:, ds(i, 1)],
            OP.mult, OP.add,
        )


def _emit_transpose(nc, pool, dst, src, ident):
    """dst = 8x8 block transpose of src; both [P, 8, 1024] (bf16)."""
    for ca in range(8):
        for cb in range(8):
            pt = pool.tile([P, P], src.dtype, tag="tp", bufs=4, name="tp")
            nc.tensor.transpose(pt[:], src[:, ca, ts(cb, P)], ident[:])
            nc.vector.tensor_copy(dst[:, cb, ts(ca, P)], pt[:])


def build_program(ln1a, ln1b, ln2a, ln2b, mask_all_ones):
    import os
    phase_stop = int(os.environ.get("BASSK_PHASE", "9"))
    nc = bacc.Bacc("TRN2", target_bir_lowering=False, debug=False)

    x_d = nc.dram_tensor("x", (S, D), F32, kind="ExternalInput").ap()
    wqT_d = nc.dram_tensor("wqT", (D, D), BF, kind="ExternalInput").ap()
    woT_d = nc.dram_tensor("woT", (D, D), BF, kind="ExternalInput").ap()
    w1T_d = nc.dram_tensor("w1T", (D, DFF), BF, kind="ExternalInput").ap()
    w2T_d = nc.dram_tensor("w2T", (DFF, D), BF, kind="ExternalInput").ap()
    bq_d = nc.dram_tensor("bq_v", (P, DC), F32, kind="ExternalInput").ap()
    b1_d = nc.dram_tensor("b1_v", (P, FC), F32, kind="ExternalInput").ap()
    bo_d = nc.dram_tensor("bo_rep", (P, D), F32, kind="ExternalInput").ap()
    b2_d = nc.dram_tensor("b2_rep", (P, D), F32, kind="ExternalInput").ap()
    if not mask_all_ones:
        m01_d = nc.dram_tensor("m01_v", (P, SC), F32, kind="ExternalInput").ap()
    out_d = nc.dram_tensor("out", (S, D), F32, kind="ExternalOutput").ap()

    x_r = x_d.rearrange("(sc p) d -> sc p d", p=P)
    wqT_r = wqT_d.rearrange("(kc p) o -> kc p o", p=P)
    woT_r = woT_d.rearrange("(oc p) d -> oc p d", p=P)
    w1_batched = w1T_d.rearrange("(dc p) f -> p dc f", p=P)
    w2_batched = w2T_d.rearrange("(fc p) d -> p fc d", p=P)
    out_r = out_d.rearrange("(sc p) d -> sc p d", p=P)

    with tile.TileContext(nc) as tc, ExitStack() as st:
        arena = st.enter_context(tc.tile_pool(name="arena", bufs=1))
        small = st.enter_context(tc.tile_pool(name="small", bufs=1))

        # ---- constants ----
        ident_b = small.tile([P, P], BF, name="ident_b")
        make_identity(nc, ident_b[:])
        ones_b = small.tile([P, P], BF, name="ones_b")
        nc.gpsimd.memset(ones_b[:], 1.0)
        ebias = small.tile([P, 1], F32, name="ebias")
        nc.gpsimd.memset(ebias[:], EXP_SHIFT)
        bq_sb = small.tile([P, DC], F32, name="bq_sb")
        nc.sync.dma_start(bq_sb[:], bq_d)
        b1_sb = small.tile([P, FC], F32, name="b1_sb")
        nc.sync.dma_start(b1_sb[:], b1_d)
        bo_rep = small.tile([P, D], F32, name="bo_rep")
        nc.sync.dma_start(bo_rep[:], bo_d)
        b2_rep = small.tile([P, D], F32, name="b2_rep")
        nc.sync.dma_start(b2_rep[:], b2_d)
        if not mask_all_ones:
            m01_sb = small.tile([P, SC], F32, name="m01_sb")
            nc.sync.dma_start(m01_sb[:], m01_d)

        dma_engines = [nc.sync, nc.scalar, nc.gpsimd]

        # ---- phase A inputs ----
        xt = arena.tile([P, SC, D], F32, tag="xt_h1", name="xt")
        for sc in range(SC):
            dma_engines[sc % 3].dma_start(xt[:, sc], x_r[sc])
        qq = arena.tile([P, 2 * DC, S], BF, tag="qq_out", name="qq")
        qT = qq[:, 0:DC]        # [o%P, oc, s]
        qh = qq[:, DC:2 * DC]   # [s%P, sc, o]
        n1 = arena.tile([P, SC, D], BF, tag="n1_ctx", name="n1")
        n1T = arena.tile([P, DC, S], BF, tag="n1T_woT", name="n1T")
        wq_sb = arena.tile([P, DC, D], BF, tag="wq_res1", name="wq_sb")
        for kc in range(DC):
            dma_engines[(kc + 1) % 3].dma_start(wq_sb[:, kc], wqT_r[kc])

        # ================= phase A: LN1, q projection, transposes ============
        _emit_layernorm(nc, small, xt, n1, ln1a, ln1b, "1", range(SC))
        with tc.tile_pool(name="psA", bufs=1, space="PSUM") as psA:
            _emit_transpose(nc, psA, n1T, n1, ident_b)
            for oc in range(DC):
                pb = [psA.tile([P, 512], F32, tag="qps", bufs=2, name="qps")
                      for _ in range(2)]
                for kc in range(DC):
                    for b in range(2):
                        nc.tensor.matmul(
                            pb[b][:], wq_sb[:, kc, ts(oc, P)],
                            n1T[:, kc, ds(512 * b, 512)],
                            start=(kc == 0), stop=(kc == DC - 1),
                        )
                for b in range(2):
                    nc.vector.tensor_scalar(
                        qT[:, oc, ds(512 * b, 512)], pb[b][:],
                        bq_sb[:, ds(oc, 1)], None, OP.add,
                    )
            for oc in range(DC):
                for sc in range(SC):
                    pt = psA.tile([P, P], BF, tag="tp", bufs=4, name="tp")
                    nc.tensor.transpose(pt[:], qT[:, oc, ts(sc, P)], ident_b[:])
                    nc.vector.tensor_copy(qh[:, sc, ts(oc, P)], pt[:])

        if phase_stop <= 1:
            for sc in range(SC):
                dt_ = arena.tile([P, D], F32, tag="dump", bufs=2, name="dump")
                nc.vector.tensor_copy(dt_[:], qh[:, sc])
                nc.sync.dma_start(out_r[sc], dt_[:])
            nc.compile()
            return nc

        # persistent across the pipelined halves
        ctxT = arena.tile([P, DC, S], BF, tag="n1_ctx", name="ctxT")
        woT_sb = arena.tile([P, DC, D], BF, tag="n1T_woT", name="woT_sb")
        for oc in range(DC):
            nc.gpsimd.dma_start(woT_sb[:, oc], woT_r[oc])
        res1 = arena.tile([P, SC, D], F32, tag="wq_res1", name="res1")
        out_sb = None

        # ============ attention / wo / LN2 / FFN pipelined by query halves ===
        #
        # exp (ScalarE) is the serial bottleneck of attention; splitting all
        # loops over queries lets FFN(half 0) matmuls run under the exps of
        # attention(half 1).  Z = column sums of E come from ones-matmuls
        # (exact, mask-friendly), replicated across psum partitions so the
        # 1/Z normalization is a plain tensor_tensor against the ctx psum.

        def attn_half(psT, half):
            for hp in range(H // 2):
                cp = psT.tile([P, 512], F32, tag="ctxp", bufs=2, name="ctxp")
                zp = psT.tile([P, 512], F32, tag="zps", bufs=2, name="zps")
                ec = arena.tile([P, SC, 2 * 512], BF, tag="EC", bufs=2,
                                name="ec")
                for c in range(SC):
                    sp = psT.tile([P, 1024], F32, tag="scp", bufs=1, name="scp")
                    for hl in range(2):
                        lo = hl * DK
                        nc.tensor.matmul(
                            sp[:, ds(hl * 512, 512)],
                            qT[ds(lo, DK), hp, ts(c, P)],
                            qT[ds(lo, DK), hp, ds(512 * half, 512)],
                            start=True, stop=True,
                            tile_position=(lo, 0),
                        )
                    nc.scalar.activation(
                        ec[:, c], sp[:], AF.Exp, bias=ebias[:], scale=0.125,
                    )
                    if not mask_all_ones:
                        nc.vector.tensor_scalar_mul(
                            ec[:, c], ec[:, c], m01_sb[:, ds(c, 1)],
                        )
                    for hl in range(2):
                        nc.tensor.matmul(
                            zp[ds(hl * DK, DK), :],
                            ones_b[:, ds(hl * DK, DK)],
                            ec[:, c, ds(hl * 512, 512)],
                            start=(c == 0), stop=(c == SC - 1),
                            tile_position=(0, hl * DK),
                            skip_group_check=True,
                        )
                        nc.tensor.matmul(
                            cp[ds(hl * DK, DK), :],
                            qh[:, c, ds(hp * P + hl * DK, DK)],
                            ec[:, c, ds(hl * 512, 512)],
                            start=(c == 0), stop=(c == SC - 1),
                            tile_position=(0, hl * DK),
                            skip_group_check=True,
                        )
                rz = arena.tile([P, 512], F32, tag="rzab", bufs=2, name="rz")
                nc.vector.reciprocal(rz[:], zp[:])
                nc.vector.tensor_mul(
                    ctxT[:, hp, ds(512 * half, 512)], cp[:], rz[:],
                )

        def wo_half(psW, half):
            for sl in range(SC // 2):
                sc = half * (SC // 2) + sl
                xre = arena.tile([P, D], F32, tag="xre", bufs=2, name="xre")
                dma_engines[sl % 3].dma_start(xre[:], x_r[sc])
                for dh in range(2):
                    wp = psW.tile([P, 512], F32, tag="wops", bufs=1,
                                  name="wops")
                    for oc in range(DC):
                        nc.tensor.matmul(
                            wp[:], ctxT[:, oc, ts(sc, P)],
                            woT_sb[:, oc, ds(512 * dh, 512)],
                            start=(oc == 0), stop=(oc == DC - 1),
                        )
                    nc.vector.tensor_add(
                        res1[:, sc, ds(512 * dh, 512)], wp[:],
                        xre[:, ds(512 * dh, 512)],
                    )
                    nc.vector.tensor_add(
                        res1[:, sc, ds(512 * dh, 512)],
                        res1[:, sc, ds(512 * dh, 512)],
                        bo_rep[:, ds(512 * dh, 512)],
                    )

        def ln2_half(psB, half):
            n2h = arena.tile([P, SC // 2, D], BF, tag="n2h", bufs=1,
                             name="n2h")
            chunks = range(half * (SC // 2), (half + 1) * (SC // 2))
            _emit_layernorm(nc, small, res1, n2h, ln2a, ln2b, f"2h{half}",
                            chunks)
            n2Th = arena.tile([P, DC, 512], BF, tag="n2th", bufs=1,
                              name="n2Th")
            for ca in range(SC // 2):
                for cb in range(DC):
                    pt = psB.tile([P, P], BF, tag="f1tp", bufs=1, name="f1tp")
                    nc.tensor.transpose(pt[:], n2h[:, ca, ts(cb, P)],
                                        ident_b[:])
                    nc.vector.tensor_copy(n2Th[:, cb, ts(ca, P)], pt[:])
            for sc in chunks:
                nc.vector.tensor_add(res1[:, sc], res1[:, sc], b2_rep[:])
            return n2Th

        def ffn1_half(psB, wsp, half, n2Th):
            h1 = arena.tile([P, FC, 512], BF, tag="xt_h1", name="h1")
            for fc in range(FC):
                wts = wsp.tile([P, DC, P], BF, tag="w1s", bufs=2, name="w1s")
                dma_engines[fc % 3].dma_start(
                    wts[:], w1_batched[:, :, ts(fc, P)])
                fp = psB.tile([P, 512], F32, tag="f1tp", bufs=1, name="f1ps")
                for dc in range(DC):
                    nc.tensor.matmul(
                        fp[:], wts[:, dc], n2Th[:, dc, :],
                        start=(dc == 0), stop=(dc == DC - 1),
                    )
                nc.vector.tensor_scalar(
                    h1[:, fc], fp[:], b1_sb[:, ds(fc, 1)], 0.0,
                    OP.add, OP.max,
                )
            return h1

        def ffn2_half(psF2, wsp, half, h1):
            nonlocal out_sb
            if out_sb is None:
                out_sb = arena.tile([P, SC, D], F32, tag="qq_out",
                                    name="out_sb")
            for dh in range(2):
                ops = [psF2.tile([P, 512], F32, tag="f2ps", bufs=4,
                                 name="f2ps") for _ in range(4)]
                for fc2 in range(FC // 2):
                    w2t = wsp.tile([P, 2, 512], BF, tag="w2s", bufs=3,
                                   name="w2s")
                    dma_engines[fc2 % 3].dma_start(
                        w2t[:],
                        w2_batched[:, ds(2 * fc2, 2), ds(512 * dh, 512)])
                    for fi in range(2):
                        fc = 2 * fc2 + fi
                        for sl in range(4):
                            nc.tensor.matmul(
                                ops[sl][:], h1[:, fc, ts(sl, P)], w2t[:, fi],
                                start=(fc == 0), stop=(fc == FC - 1),
                            )
                for sl in range(4):
                    sc = half * 4 + sl
                    nc.vector.tensor_add(
                        out_sb[:, sc, ds(512 * dh, 512)], ops[sl][:],
                        res1[:, sc, ds(512 * dh, 512)],
                    )
                    dma_engines[sl % 2].dma_start(
                        out_r[sc][:, ds(512 * dh, 512)],
                        out_sb[:, sc, ds(512 * dh, 512)],
                    )

        with tc.tile_pool(name="psB", bufs=1, space="PSUM") as psB, \
             tc.tile_pool(name="wstream", bufs=1) as wsp:
            with tc.tile_pool(name="psAtt", bufs=1, space="PSUM") as psT, \
                 tc.tile_pool(name="psWo", bufs=1, space="PSUM") as psW:
                attn_half(psT, 0)
                wo_half(psW, 0)
                n2Th0 = ln2_half(psB, 0)
                h10 = ffn1_half(psB, wsp, 0, n2Th0)
                attn_half(psT, 1)
                wo_half(psW, 1)
                n2Th1 = ln2_half(psB, 1)
            with tc.tile_pool(name="psF2", bufs=1, space="PSUM") as psF2:
                ffn2_half(psF2, wsp, 0, h10)
                h11 = ffn1_half(psB, wsp, 1, n2Th1)
                ffn2_half(psF2, wsp, 1, h11)

    nc.compile()
    return nc


def _prep_inputs(inputs):
    f32 = lambda a: np.ascontiguousarray(np.asarray(a, dtype=np.float32))
    bfT = lambda a: np.ascontiguousarray(
        np.asarray(a, dtype=np.float32).T.astype(ml_dtypes.bfloat16))
    x = f32(inputs["x"])                      # [B, S, D]
    mask = np.asarray(inputs["src_mask"])     # [B, 1, 1, S] int32
    wqT = bfT(inputs["wq"])                   # [D, D] (in, out)
    woT = bfT(inputs["wo"])
    w1T = bfT(inputs["w1"])                   # [D, DFF]
    w2T = bfT(inputs["w2"])                   # [DFF, D]
    bq_v = np.ascontiguousarray(f32(inputs["bq"]).reshape(DC, P).T)
    b1_v = np.ascontiguousarray(f32(inputs["b1"]).reshape(FC, P).T)
    bo_rep = np.ascontiguousarray(np.tile(f32(inputs["bo"]), (P, 1)))
    b2_rep = np.ascontiguousarray(np.tile(f32(inputs["b2"]), (P, 1)))
    scal = lambda k: float(np.asarray(inputs[k]).reshape(-1)[0])
    ln = (scal("ln1_a"), scal("ln1_b"), scal("ln2_a"), scal("ln2_b"))
    mask_all_ones = bool((mask != 0).all())

    shared = dict(wqT=wqT, woT=woT, w1T=w1T, w2T=w2T, bq_v=bq_v, b1_v=b1_v,
                  bo_rep=bo_rep, b2_rep=b2_rep)
    in_maps = []
    for b in range(NB):
        m = dict(shared)
        m["x"] = np.ascontiguousarray(x[b])
        if not mask_all_ones:
            m01 = (mask[b].reshape(S) != 0).astype(np.float32)
            m["m01_v"] = np.ascontiguousarray(m01.reshape(SC, P).T)
            m["m01_rep"] = np.ascontiguousarray(np.tile(m01, (P, 1)))
        in_maps.append(m)
    return in_maps, ln, mask_all_ones


last_nc = None
last_in_maps = None


def kernel(**inputs):
    global last_nc, last_in_maps
    in_maps, ln, mask_all_ones = _prep_inputs(inputs)
    nc = build_program(*ln, mask_all_ones)
    last_nc, last_in_maps = nc, in_maps
    res = bass_utils.run_bass_kernel_spmd(
        nc, in_maps, core_ids=list(range(NB)), trace=False,
    )
    out = np.stack([np.asarray(res.results[b]["out"]) for b in range(NB)])
    return out.astype(np.float32)



# revision 9
# speedup vs baseline: 1.1756x; 1.1756x over previous
"""Trainium2 Bass kernel for nn_EncoderBlock (dense transformer encoder block).

Data parallel: batch B=8 across 8 NeuronCores, one element per core.

v2 design vs v1 baseline (685us):
  - scores matmuls in fp8 DoubleRow (zero-padded K=64 subtile): 2x.
  - ctx computed "flipped" (out = [queries, feats]) in fp8 DoubleRow with a
    fused ones-column producing the softmax normalizer Z in the same psum
    tile; per-partition normalize via reciprocal_approx_fast + tensor_scalar.
    Kills the ones-matmul for Z and the replicated [128,512] reciprocal.
  - wo in fp8 DoubleRow; wo weights pre-scaled x64 host-side and ctx scaled
    x16 on-chip (fp8 subnormal avoidance), un-scaled by 1/1024 in the
    residual add.
  - qproj per head-pair pipelined under the attention-half-0 exp stream;
    ffn1(half 0) interleaved under the attention-half-1 exp stream.
  - LN sum/sum-of-squares both via ScalarE ACT accumulate (Copy / Square).
FFN and qproj matmuls stay bf16 (fp8 there would blow the 2e-2 error gate).
"""

import sys

sys.path.insert(0, "/opt/trn_rl_repo")

import numpy as np
import ml_dtypes
from contextlib import ExitStack

import concourse.bass as bass
import concourse.tile as tile
from concourse import bacc, mybir
from concourse import bass_utils
from concourse.bass import ts, ds
from concourse.masks import make_identity

BF = mybir.dt.bfloat16
F32 = mybir.dt.float32
FP8 = mybir.dt.float8e4
AF = mybir.ActivationFunctionType
OP = mybir.AluOpType
DR = mybir.MatmulPerfMode.DoubleRow

P = 128
S = 1024          # sequence length per core
D = 1024          # d_model
H = 16            # heads
DK = 64           # head dim
DFF = 4096
NB = 8            # batch = number of cores
SC = S // P       # 8 sequence chunks
DC = D // P       # 8 feature chunks
FC = DFF // P     # 32 ff chunks
EPS = 1e-6
EXP_SHIFT = -2.0   # constant shift inside exp; cancels in softmax ratio
CTX_SCALE = 16.0   # on-chip scale of v (=q) into qh8e; keeps ctx out of
WO_SCALE = 64.0    # fp8 subnormal range.  woT8 = fp8(64*wo) host-side.
OUT_SCALE = 1.0 / (CTX_SCALE * WO_SCALE)
HW = 66            # per-head stride in qh8e: 64 feats + Z-ones col + pad

last_exec_time_ns = None


def _emit_ln_chunk(nc, small, x_ap, out_ap, scratch_ap, alpha, beta):
    """Bessel-corrected LN of one [P, D] chunk, stats per token on partitions.
    n = (x - mu)/(std + eps)*alpha + beta.  Sum and sum-of-squares come from
    ScalarE ACT accumulate (Copy / Square) writing scratch_ap; scratch is
    overwritten by the final apply (out_ap may alias scratch_ap)."""
    s1 = small.tile([P, 1], F32, tag="ln_s1", bufs=3, name="ln_s1")
    sq = small.tile([P, 1], F32, tag="ln_sq", bufs=3, name="ln_sq")
    mu = small.tile([P, 1], F32, tag="ln_mu", bufs=3, name="ln_mu")
    var = small.tile([P, 1], F32, tag="ln_var", bufs=3, name="ln_var")
    tmp = small.tile([P, 1], F32, tag="ln_tmp", bufs=3, name="ln_tmp")
    s0 = small.tile([P, 1], F32, tag="ln_s0", bufs=3, name="ln_s0")
    tc_ = small.tile([P, 1], F32, tag="ln_tc", bufs=3, name="ln_tc")
    uc_ = small.tile([P, 1], F32, tag="ln_uc", bufs=3, name="ln_uc")

    nc.scalar.activation(scratch_ap, x_ap, AF.Copy, accum_out=s1[:])
    nc.scalar.activation(scratch_ap, x_ap, AF.Square, accum_out=sq[:])
    nc.vector.tensor_scalar_mul(mu[:], s1[:], 1.0 / D)
    nc.vector.tensor_mul(tmp[:], mu[:], mu[:])
    nc.vector.tensor_scalar_mul(var[:], sq[:], 1.0 / (D - 1))
    nc.vector.tensor_scalar_mul(tmp[:], tmp[:], float(D) / (D - 1))
    nc.vector.tensor_sub(var[:], var[:], tmp[:])
    # std = sqrt(var): ACT sqrt + one Newton step  s1 = 0.5*(s0 + var/s0)
    nc.scalar.activation(s0[:], var[:], AF.Sqrt)
    nc.vector.reciprocal(tmp[:], s0[:])
    nc.vector.tensor_mul(tmp[:], tmp[:], var[:])
    nc.vector.tensor_add(tmp[:], tmp[:], s0[:])
    nc.vector.tensor_scalar(tmp[:], tmp[:], 0.5, EPS, OP.mult, OP.add)
    nc.vector.reciprocal(tmp[:], tmp[:])                 # 1/(std+eps)
    nc.vector.tensor_scalar_mul(tc_[:], tmp[:], float(alpha))
    nc.vector.tensor_mul(tmp[:], mu[:], tc_[:])
    nc.vector.tensor_scalar(uc_[:], tmp[:], -1.0, float(beta), OP.mult, OP.add)
    nc.vector.tensor_scalar(out_ap, x_ap, tc_[:], uc_[:], OP.mult, OP.add)


def build_program(ln1a, ln1b, ln2a, ln2b, mask_all_ones):
    nc = bacc.Bacc("TRN2", target_bir_lowering=False, debug=False)

    x_d = nc.dram_tensor("x", (S, D), F32, kind="ExternalInput").ap()
    wqT_d = nc.dram_tensor("wqT", (D, D), BF, kind="ExternalInput").ap()
    woT8_d = nc.dram_tensor("woT8", (D, D), FP8, kind="ExternalInput").ap()
    w1T_d = nc.dram_tensor("w1T", (D, DFF), BF, kind="ExternalInput").ap()
    w2T_d = nc.dram_tensor("w2T", (DFF, D), BF, kind="ExternalInput").ap()
    bq_d = nc.dram_tensor("bq_v", (P, DC), F32, kind="ExternalInput").ap()
    b1_d = nc.dram_tensor("b1_v", (P, FC), F32, kind="ExternalInput").ap()
    bo_d = nc.dram_tensor("bo_rep", (P, D), F32, kind="ExternalInput").ap()
    b2_d = nc.dram_tensor("b2_rep", (P, D), F32, kind="ExternalInput").ap()
    if not mask_all_ones:
        m01_d = nc.dram_tensor("m01_v", (P, SC), F32, kind="ExternalInput").ap()
    out_d = nc.dram_tensor("out", (S, D), F32, kind="ExternalOutput").ap()

    x_r = x_d.rearrange("(sc p) d -> sc p d", p=P)
    wqT_r = wqT_d.rearrange("(kc p) o -> kc p o", p=P)
    woT8_r = woT8_d.rearrange("(oc p) d -> oc p d", p=P)
    w1_batched = w1T_d.rearrange("(dc p) f -> p dc f", p=P)
    w2_batched = w2T_d.rearrange("(fc p) d -> p fc d", p=P)
    out_r = out_d.rearrange("(sc p) d -> sc p d", p=P)

    with tile.TileContext(nc) as tc, ExitStack() as st:
        arena = st.enter_context(tc.tile_pool(name="arena", bufs=1))
        small = st.enter_context(tc.tile_pool(name="small", bufs=1))

        # ---- constants ----
        ident_b = small.tile([P, P], BF, name="ident_b")
        make_identity(nc, ident_b[:])
        ident8 = small.tile([P, P], FP8, name="ident8")
        make_identity(nc, ident8[:])
        ebias = small.tile([P, 1], F32, name="ebias")
        nc.gpsimd.memset(ebias[:], EXP_SHIFT)
        bq_sb = small.tile([P, DC], F32, name="bq_sb")
        nc.sync.dma_start(bq_sb[:], bq_d)
        b1_sb = small.tile([P, FC], F32, name="b1_sb")
        nc.sync.dma_start(b1_sb[:], b1_d)
        bo_rep = small.tile([P, D], F32, name="bo_rep")
        nc.gpsimd.dma_start(bo_rep[:], bo_d)
        b2_rep = small.tile([P, D], F32, name="b2_rep")
        nc.gpsimd.dma_start(b2_rep[:], b2_d)
        if not mask_all_ones:
            m01_sb = small.tile([P, SC], F32, name="m01_sb")
            nc.sync.dma_start(m01_sb[:], m01_d)

        # ---- persistent sbuf tiles ----
        qT8 = arena.tile([P, DC, 2, S], FP8, tag="qT8", name="qT8")
        qh8e = arena.tile([P, SC, H * HW], FP8, tag="qh8e", name="qh8e")
        ctxN = arena.tile([P, SC, D], FP8, tag="ctxN", name="ctxN")
        res1 = arena.tile([P, SC, D], F32, tag="res1", name="res1")
        n1T = arena.tile([P, DC, S], BF, tag="n1T_ctxT8", name="n1T")
        wq_sb = arena.tile([P, DC, D], BF, tag="wq_n2t", bufs=2, name="wq_sb")
        woT8_sb = arena.tile([P, DC, D], FP8, tag="woT8", name="woT8_sb")

        # zero the DoubleRow pad subtile of qT8 and the Z/pad cols of qh8e
        for oc in range(DC):
            eng = nc.vector if oc % 2 == 0 else nc.gpsimd
            eng.memset(qT8[:, oc, 1, :], 0.0)
        for h in range(H):
            nc.gpsimd.memset(qh8e[:, :, ds(h * HW + DK, 1)], 1.0)
            nc.gpsimd.memset(qh8e[:, :, ds(h * HW + DK + 1, 1)], 0.0)

        for kc in range(DC):
            (nc.sync if kc % 2 == 0 else nc.gpsimd).dma_start(
                wq_sb[:, kc], wqT_r[kc])
        for oc in range(DC):
            nc.gpsimd.dma_start(woT8_sb[:, oc], woT8_r[oc])

        # =========== phase 1: LN1 streamed per chunk + n1T transposes =======
        with tc.tile_pool(name="ps1", bufs=1, space="PSUM") as ps1:
            for sc in range(SC):
                xts = arena.tile([P, D], F32, tag="xts", bufs=2, name="xts")
                nc.sync.dma_start(xts[:], x_r[sc])
                n1s = arena.tile([P, D], BF, tag="n1s", bufs=2, name="n1s")
                _emit_ln_chunk(nc, small, xts[:], n1s[:], n1s[:], ln1a, ln1b)
                for cb in range(DC):
                    tpB = ps1.tile([P, P], BF, tag="tpB", bufs=4, name="tpB")
                    nc.tensor.transpose(tpB[:], n1s[:, ts(cb, P)], ident_b[:])
                    nc.vector.tensor_copy(n1T[:, cb, ts(sc, P)], tpB[:])

        # attention helpers ---------------------------------------------------
        def qproj_head(psQ, ps_tp8, hp):
            """q projection for feature chunk oc=hp -> qT8 + qh8e slices."""
            for b in range(2):
                pb = psQ.tile([P, 512], F32, tag="pb", bufs=1, name="pb")
                for kc in range(DC):
                    nc.tensor.matmul(
                        pb[:], wq_sb[:, kc, ts(hp, P)],
                        n1T[:, kc, ds(512 * b, 512)],
                        start=(kc == 0), stop=(kc == DC - 1),
                    )
                nc.vector.tensor_scalar(
                    qT8[:, hp, 0, ds(512 * b, 512)], pb[:],
                    bq_sb[:, ds(hp, 1)], None, OP.add,
                )
            for sc in range(SC):
                tp8 = ps_tp8.tile([P, P, 2], FP8, tag="tp8", bufs=1,
                                  name="tp8")
                nc.tensor.transpose(
                    tp8[:, :, ds(0, 1)], qT8[:, hp, 0, ts(sc, P)], ident8[:])
                for hl in range(2):
                    nc.vector.tensor_scalar_mul(
                        qh8e[:, sc, ds((2 * hp + hl) * HW, DK)],
                        tp8[:, ds(hl * DK, DK), 0], CTX_SCALE,
                    )

        def attn_head(psS, psC, hp, half):
            """scores+exp+ctx for head pair hp, query half `half`."""
            ec8 = arena.tile([P, SC, 1024], FP8, tag="ec8", bufs=2, name="ec8")
            for c in range(SC):
                sp = psS.tile([P, 1024], F32, tag="sp", bufs=2, name="sp")
                for hl in range(2):
                    lo = hl * DK
                    nc.tensor.matmul(
                        sp[:, ds(hl * 512, 512)],
                        qT8[ds(lo, DK), hp, :, ts(c, P)],
                        qT8[ds(lo, DK), hp, :, ds(512 * half, 512)],
                        start=True, stop=True,
                        perf_mode=DR,
                        tile_position=(lo, 0),
                    )
                nc.scalar.activation(
                    ec8[:, c], sp[:], AF.Exp, bias=ebias[:], scale=0.125,
                )
                if not mask_all_ones:
                    nc.vector.tensor_scalar_mul(
                        ec8[:, c], ec8[:, c], m01_sb[:, ds(c, 1)],
                    )
            for hl in range(2):
                h = 2 * hp + hl
                for qc in range(4):
                    cxp = psC.tile([P, 128], F32, tag="cxp", bufs=2,
                                   name="cxp")
                    for ci in range(SC // 2):
                        nc.tensor.matmul(
                            cxp[:, ds(0, HW)],
                            ec8[:, ds(2 * ci, 2),
                                ds(hl * 512 + qc * P, P)],
                            qh8e[:, ds(2 * ci, 2), ds(h * HW, HW)],
                            start=(ci == 0), stop=(ci == SC // 2 - 1),
                            perf_mode=DR,
                        )
                    rz = small.tile([P, 1], F32, tag="rz", bufs=3, name="rz")
                    nc.vector.reciprocal_approx_fast(rz[:], cxp[:, ds(DK, 1)])
                    nc.vector.tensor_scalar(
                        ctxN[:, half * 4 + qc, ds(h * DK, DK)],
                        cxp[:, ds(0, DK)], rz[:], None, OP.mult,
                    )

        def ctx_transpose_half(ps_tp, half):
            for sl in range(4):
                sc = half * 4 + sl
                for oc in range(DC):
                    tpC8 = ps_tp.tile([P, P, 2], FP8, tag="tpX", bufs=1,
                                      name="tpC8")
                    nc.tensor.transpose(
                        tpC8[:, :, ds(0, 1)], ctxN[:, sc, ts(oc, P)],
                        ident8[:])
                    nc.vector.tensor_copy(
                        ctxT8_t[:, oc, ts(sc, P)], tpC8[:, :, 0])

        def wo_half(psW, half):
            for sl in range(4):
                sc = half * 4 + sl
                xre = arena.tile([P, D], F32, tag="xre", bufs=1, name="xre")
                nc.sync.dma_start(xre[:], x_r[sc])
                for dh in range(2):
                    wp = psW.tile([P, 512], F32, tag="wp", bufs=1, name="wp")
                    for oi in range(DC // 2):
                        nc.tensor.matmul(
                            wp[:],
                            ctxT8_t[:, ds(2 * oi, 2), ts(sc, P)],
                            woT8_sb[:, ds(2 * oi, 2), ds(512 * dh, 512)],
                            start=(oi == 0), stop=(oi == DC // 2 - 1),
                            perf_mode=DR,
                        )
                    nc.vector.scalar_tensor_tensor(
                        res1[:, sc, ds(512 * dh, 512)], wp[:], OUT_SCALE,
                        xre[:, ds(512 * dh, 512)], OP.mult, OP.add,
                    )
                    nc.vector.tensor_add(
                        res1[:, sc, ds(512 * dh, 512)],
                        res1[:, sc, ds(512 * dh, 512)],
                        bo_rep[:, ds(512 * dh, 512)],
                    )

        def ln2_half(ps_tp, half, n2Th):
            for sl in range(4):
                sc = half * 4 + sl
                n2s = arena.tile([P, D], BF, tag="n2s", bufs=2, name="n2s")
                _emit_ln_chunk(nc, small, res1[:, sc], n2s[:], n2s[:],
                               ln2a, ln2b)
                for cb in range(DC):
                    tpC = ps_tp.tile([P, P], BF, tag="tpX", bufs=1,
                                     name="tpC")
                    nc.tensor.transpose(tpC[:], n2s[:, ts(cb, P)], ident_b[:])
                    nc.vector.tensor_copy(n2Th[:, cb, ts(sl, P)], tpC[:])
                nc.vector.tensor_add(res1[:, sc], res1[:, sc], b2_rep[:])

        def ffn1_chunk(psF, wsp, n2Th, h1, fc, relu_on_scalar):
            wts = wsp.tile([P, DC, P], BF, tag="w1s", bufs=2, name="w1s")
            (nc.sync if fc % 2 == 0 else nc.gpsimd).dma_start(
                wts[:], w1_batched[:, :, ts(fc, P)])
            fp = psF.tile([P, 512], F32, tag="f1ps", bufs=2, name="f1ps")
            for dc in range(DC):
                nc.tensor.matmul(
                    fp[:], wts[:, dc], n2Th[:, dc, :],
                    start=(dc == 0), stop=(dc == DC - 1),
                )
            if relu_on_scalar:
                nc.scalar.activation(
                    h1[:, fc], fp[:], AF.Relu, bias=b1_sb[:, ds(fc, 1)],
                )
            else:
                nc.vector.tensor_scalar(
                    h1[:, fc], fp[:], b1_sb[:, ds(fc, 1)], 0.0,
                    OP.add, OP.max,
                )

        def ffn2_half(psF2, wsp, half, h1):
            for dh in range(2):
                ops = [psF2.tile([P, 512], F32, tag="f2ps", bufs=4,
                                 name="f2ps") for _ in range(4)]
                for fc2 in range(FC // 2):
                    w2t = wsp.tile([P, 2, 512], BF, tag="w2s", bufs=2,
                                   name="w2s")
                    (nc.sync if fc2 % 2 == 0 else nc.scalar).dma_start(
                        w2t[:],
                        w2_batched[:, ds(2 * fc2, 2), ds(512 * dh, 512)])
                    for fi in range(2):
                        fc = 2 * fc2 + fi
                        for sl in range(4):
                            nc.tensor.matmul(
                                ops[sl][:], h1[:, fc, ts(sl, P)], w2t[:, fi],
                                start=(fc == 0), stop=(fc == FC - 1),
                            )
                for sl in range(4):
                    sc = half * 4 + sl
                    nc.vector.tensor_add(
                        res1[:, sc, ds(512 * dh, 512)], ops[sl][:],
                        res1[:, sc, ds(512 * dh, 512)],
                    )
                    (nc.gpsimd if sl % 2 == 0 else nc.scalar).dma_start(
                        out_r[sc][:, ds(512 * dh, 512)],
                        res1[:, sc, ds(512 * dh, 512)],
                    )

        # ================== phase 2: qproj pipeline + attention half 0 ======
        ctxT8_t = None
        with tc.tile_pool(name="wstream", bufs=1) as wsp:
            with tc.tile_pool(name="psSp", bufs=1, space="PSUM") as psS, \
                 tc.tile_pool(name="psCx", bufs=1, space="PSUM") as psC:
                with tc.tile_pool(name="psQ", bufs=1, space="PSUM") as psQ:
                    for hp in range(DC):
                        qproj_head(psQ, psQ, hp)
                        if hp >= 1:
                            attn_head(psS, psC, hp - 1, 0)
                    attn_head(psS, psC, DC - 1, 0)

                # ========= phase 3: ctxT8(0), wo(0), ln2(0) =================
                ctxT8_t = arena.tile([P, DC, S], FP8, tag="n1T_ctxT8",
                                     name="ctxT8")
                n2Th0 = arena.tile([P, DC, 512], BF, tag="wq_n2t",
                                   bufs=2, name="n2Th0")
                with tc.tile_pool(name="ps3", bufs=1, space="PSUM") as ps3:
                    ctx_transpose_half(ps3, 0)
                    wo_half(ps3, 0)
                    ln2_half(ps3, 0, n2Th0)

                # ==== phase 4: attention half 1 with ffn1(0) interleaved ====
                h1a = arena.tile([P, FC, 512], BF, tag="xt_h1", name="h1a")
                with tc.tile_pool(name="ps4", bufs=1, space="PSUM") as ps4:
                    for hp in range(DC):
                        attn_head(psS, psC, hp, 1)
                        for fc in range(4 * hp, 4 * hp + 4):
                            ffn1_chunk(ps4, wsp, n2Th0, h1a, fc, False)

            # =========== phase 5: ctxT8(1), wo(1), ln2(1) ===================
            n2Th1 = arena.tile([P, DC, 512], BF, tag="wq_n2t", bufs=2,
                               name="n2Th1")
            with tc.tile_pool(name="ps5", bufs=1, space="PSUM") as ps5:
                ctx_transpose_half(ps5, 1)
                wo_half(ps5, 1)
                ln2_half(ps5, 1, n2Th1)

            # ============== phases 6-7: ffn2(0), ffn1(1), ffn2(1) ===========
            with tc.tile_pool(name="psF2", bufs=1, space="PSUM") as psF2:
                ffn2_half(psF2, wsp, 0, h1a)
                h1b = arena.tile([P, FC, 512], BF, tag="xt_h1", name="h1b")
                for fc in range(FC):
                    ffn1_chunk(psF2, wsp, n2Th1, h1b, fc, True)
                ffn2_half(psF2, wsp, 1, h1b)

    nc.compile()
    return nc


def _prep_inputs(inputs):
    f32 = lambda a: np.ascontiguousarray(np.asarray(a, dtype=np.float32))
    bfT = lambda a: np.ascontiguousarray(
        np.asarray(a, dtype=np.float32).T.astype(ml_dtypes.bfloat16))
    x = f32(inputs["x"])                      # [B, S, D]
    mask = np.asarray(inputs["src_mask"])     # [B, 1, 1, S] int32
    wqT = bfT(inputs["wq"])                   # [D, D] (in, out)
    woT8 = np.ascontiguousarray(
        (np.asarray(inputs["wo"], dtype=np.float32).T * WO_SCALE)
        .astype(ml_dtypes.float8_e4m3))
    w1T = bfT(inputs["w1"])                   # [D, DFF]
    w2T = bfT(inputs["w2"])                   # [DFF, D]
    bq_v = np.ascontiguousarray(f32(inputs["bq"]).reshape(DC, P).T)
    b1_v = np.ascontiguousarray(f32(inputs["b1"]).reshape(FC, P).T)
    bo_rep = np.ascontiguousarray(np.tile(f32(inputs["bo"]), (P, 1)))
    b2_rep = np.ascontiguousarray(np.tile(f32(inputs["b2"]), (P, 1)))
    scal = lambda k: float(np.asarray(inputs[k]).reshape(-1)[0])
    ln = (scal("ln1_a"), scal("ln1_b"), scal("ln2_a"), scal("ln2_b"))
    mask_all_ones = bool((mask != 0).all())

    shared = dict(wqT=wqT, woT8=woT8, w1T=w1T, w2T=w2T, bq_v=bq_v, b1_v=b1_v,
                  bo_rep=bo_rep, b2_rep=b2_rep)
    in_maps = []
    for b in range(NB):
        m = dict(shared)
        m["x"] = np.ascontiguousarray(x[b])
        if not mask_all_ones:
            m01 = (mask[b].reshape(S) != 0).astype(np.float32)
            m["m01_v"] = np.ascontiguousarray(m01.reshape(SC, P).T)
        in_maps.append(m)
    return in_maps, ln, mask_all_ones


last_nc = None
last_in_maps = None


def kernel(**inputs):
    global last_nc, last_in_maps
    in_maps, ln, mask_all_ones = _prep_inputs(inputs)
    nc = build_program(*ln, mask_all_ones)
    last_nc, last_in_maps = nc, in_maps
    res = bass_utils.run_bass_kernel_spmd(
        nc, in_maps, core_ids=list(range(NB)), trace=False,
    )
    out = np.stack([np.asarray(res.results[b]["out"]) for b in range(NB)])
    return out.astype(np.float32)


# revision 11
# speedup vs baseline: 1.3879x; 1.1806x over previous
"""Trainium2 Bass kernel for nn_EncoderBlock (dense transformer encoder block).

Data parallel: batch B=8 across 8 NeuronCores, one element per core.

v2 design vs v1 baseline (685us):
  - scores matmuls in fp8 DoubleRow (zero-padded K=64 subtile): 2x.
  - ctx computed "flipped" (out = [queries, feats]) in fp8 DoubleRow with a
    fused ones-column producing the softmax normalizer Z in the same psum
    tile; per-partition normalize via reciprocal_approx_fast + tensor_scalar.
    Kills the ones-matmul for Z and the replicated [128,512] reciprocal.
  - wo in fp8 DoubleRow; wo weights pre-scaled x64 host-side and ctx scaled
    x16 on-chip (fp8 subnormal avoidance), un-scaled by 1/1024 in the
    residual add.
  - qproj per head-pair pipelined under the attention-half-0 exp stream;
    ffn1(half 0) interleaved under the attention-half-1 exp stream.
  - LN sum/sum-of-squares both via ScalarE ACT accumulate (Copy / Square).
FFN and qproj matmuls stay bf16 (fp8 there would blow the 2e-2 error gate).
"""

import sys

sys.path.insert(0, "/opt/trn_rl_repo")

import numpy as np
import ml_dtypes
from contextlib import ExitStack

import concourse.bass as bass
import concourse.tile as tile
from concourse import bacc, mybir
from concourse import bass_utils
from concourse.bass import ts, ds
from concourse.masks import make_identity

BF = mybir.dt.bfloat16
F32 = mybir.dt.float32
FP8 = mybir.dt.float8e4
AF = mybir.ActivationFunctionType
OP = mybir.AluOpType
DR = mybir.MatmulPerfMode.DoubleRow

P = 128
S = 1024          # sequence length per core
D = 1024          # d_model
H = 16            # heads
DK = 64           # head dim
DFF = 4096
NB = 8            # batch = number of cores
SC = S // P       # 8 sequence chunks
DC = D // P       # 8 feature chunks
FC = DFF // P     # 32 ff chunks
EPS = 1e-6
EXP_SHIFT = -2.0   # constant shift inside exp; cancels in softmax ratio
CTX_SCALE = 16.0   # on-chip scale of v (=q) into qh8e; keeps ctx out of
WO_SCALE = 64.0    # fp8 subnormal range.  woT8 = fp8(64*wo) host-side.
OUT_SCALE = 1.0 / (CTX_SCALE * WO_SCALE)
HW = 66            # per-head stride in qh8e: 64 feats + Z-ones col + pad

last_exec_time_ns = None


def _emit_ln_chunk(nc, small, x_ap, out_ap, scratch_ap, alpha, beta):
    """Bessel-corrected LN of one [P, D] chunk, stats per token on partitions.
    n = (x - mu)/(std + eps)*alpha + beta.  Sum and sum-of-squares come from
    ScalarE ACT accumulate (Copy / Square) writing scratch_ap; scratch is
    overwritten by the final apply (out_ap may alias scratch_ap)."""
    s1 = small.tile([P, 1], F32, tag="ln_s1", bufs=3, name="ln_s1")
    sq = small.tile([P, 1], F32, tag="ln_sq", bufs=3, name="ln_sq")
    mu = small.tile([P, 1], F32, tag="ln_mu", bufs=3, name="ln_mu")
    var = small.tile([P, 1], F32, tag="ln_var", bufs=3, name="ln_var")
    tmp = small.tile([P, 1], F32, tag="ln_tmp", bufs=3, name="ln_tmp")
    s0 = small.tile([P, 1], F32, tag="ln_s0", bufs=3, name="ln_s0")
    tc_ = small.tile([P, 1], F32, tag="ln_tc", bufs=3, name="ln_tc")
    uc_ = small.tile([P, 1], F32, tag="ln_uc", bufs=3, name="ln_uc")

    nc.scalar.activation(scratch_ap, x_ap, AF.Copy, accum_out=s1[:])
    nc.scalar.activation(scratch_ap, x_ap, AF.Square, accum_out=sq[:])
    nc.vector.tensor_scalar_mul(mu[:], s1[:], 1.0 / D)
    nc.vector.tensor_mul(tmp[:], mu[:], mu[:])
    nc.vector.tensor_scalar_mul(var[:], sq[:], 1.0 / (D - 1))
    nc.vector.tensor_scalar_mul(tmp[:], tmp[:], float(D) / (D - 1))
    nc.vector.tensor_sub(var[:], var[:], tmp[:])
    # std = sqrt(var): ACT sqrt + one Newton step  s1 = 0.5*(s0 + var/s0)
    nc.scalar.activation(s0[:], var[:], AF.Sqrt)
    nc.vector.reciprocal(tmp[:], s0[:])
    nc.vector.tensor_mul(tmp[:], tmp[:], var[:])
    nc.vector.tensor_add(tmp[:], tmp[:], s0[:])
    nc.vector.tensor_scalar(tmp[:], tmp[:], 0.5, EPS, OP.mult, OP.add)
    nc.vector.reciprocal(tmp[:], tmp[:])                 # 1/(std+eps)
    nc.vector.tensor_scalar_mul(tc_[:], tmp[:], float(alpha))
    nc.vector.tensor_mul(tmp[:], mu[:], tc_[:])
    nc.vector.tensor_scalar(uc_[:], tmp[:], -1.0, float(beta), OP.mult, OP.add)
    nc.vector.tensor_scalar(out_ap, x_ap, tc_[:], uc_[:], OP.mult, OP.add)


def build_program(ln1a, ln1b, ln2a, ln2b, mask_all_ones):
    nc = bacc.Bacc("TRN2", target_bir_lowering=False, debug=False)

    x_d = nc.dram_tensor("x", (S, D), F32, kind="ExternalInput").ap()
    wqT_d = nc.dram_tensor("wqT", (D, D), BF, kind="ExternalInput").ap()
    woT8_d = nc.dram_tensor("woT8", (D, D), FP8, kind="ExternalInput").ap()
    w1L_d = nc.dram_tensor("w1L", (P, FC, DC, P), BF, kind="ExternalInput").ap()
    w2L_d = nc.dram_tensor("w2L", (P, 2, FC // 2, 2, 512), BF, kind="ExternalInput").ap()
    bq_d = nc.dram_tensor("bq_v", (P, DC), F32, kind="ExternalInput").ap()
    b1_d = nc.dram_tensor("b1_v", (P, FC), F32, kind="ExternalInput").ap()
    bo_d = nc.dram_tensor("bo_rep", (P, D), F32, kind="ExternalInput").ap()
    b2_d = nc.dram_tensor("b2_rep", (P, D), F32, kind="ExternalInput").ap()
    if not mask_all_ones:
        m01_d = nc.dram_tensor("m01_v", (P, SC), F32, kind="ExternalInput").ap()
    out_d = nc.dram_tensor("out", (S, D), F32, kind="ExternalOutput").ap()

    x_r = x_d.rearrange("(sc p) d -> sc p d", p=P)
    wqT_r = wqT_d.rearrange("(kc p) o -> kc p o", p=P)
    woT8_r = woT8_d.rearrange("(oc p) d -> oc p d", p=P)
    out_r = out_d.rearrange("(sc p) d -> sc p d", p=P)

    with tile.TileContext(nc) as tc, ExitStack() as st:
        arena = st.enter_context(tc.tile_pool(name="arena", bufs=1))
        small = st.enter_context(tc.tile_pool(name="small", bufs=1))

        # ---- constants ----
        ident_b = small.tile([P, P], BF, name="ident_b")
        make_identity(nc, ident_b[:])
        ident8 = small.tile([P, P], FP8, name="ident8")
        make_identity(nc, ident8[:])
        ebias = small.tile([P, 1], F32, name="ebias")
        nc.gpsimd.memset(ebias[:], EXP_SHIFT)
        bq_sb = small.tile([P, DC], F32, name="bq_sb")
        nc.sync.dma_start(bq_sb[:], bq_d)
        b1_sb = small.tile([P, FC], F32, name="b1_sb")
        nc.sync.dma_start(b1_sb[:], b1_d)
        bo_rep = small.tile([P, D], F32, name="bo_rep")
        nc.gpsimd.dma_start(bo_rep[:], bo_d)
        b2_rep = small.tile([P, D], F32, name="b2_rep")
        nc.gpsimd.dma_start(b2_rep[:], b2_d)
        if not mask_all_ones:
            m01_sb = small.tile([P, SC], F32, name="m01_sb")
            nc.sync.dma_start(m01_sb[:], m01_d)

        # ---- persistent sbuf tiles ----
        qT8 = arena.tile([P, DC, S], FP8, tag="qT8", name="qT8")
        qh8e = arena.tile([P, SC, H * HW], FP8, tag="qh8e", name="qh8e")
        ctxN = arena.tile([P, SC, D], FP8, tag="ctxN", name="ctxN")
        res1 = arena.tile([P, SC, D], F32, tag="res1", name="res1")
        n1T = arena.tile([P, DC, S], BF, tag="n1T_ctxT8", name="n1T")
        wq_sb = arena.tile([P, DC, D], BF, tag="wq_n2t", bufs=2, name="wq_sb")
        woT8_sb = arena.tile([P, DC, D], FP8, tag="woT8", name="woT8_sb")

        # zero the Z-ones / pad cols of qh8e
        for h in range(H):
            nc.gpsimd.memset(qh8e[:, :, ds(h * HW + DK, 1)], 1.0)
            nc.gpsimd.memset(qh8e[:, :, ds(h * HW + DK + 1, 1)], 0.0)

        for kc in range(DC):
            (nc.sync if kc % 2 == 0 else nc.gpsimd).dma_start(
                wq_sb[:, kc], wqT_r[kc])
        for oc in range(DC):
            nc.gpsimd.dma_start(woT8_sb[:, oc], woT8_r[oc])

        # =========== phase 1: LN1 streamed per chunk + n1T transposes =======
        with tc.tile_pool(name="ps1", bufs=1, space="PSUM") as ps1:
            for sc in range(SC):
                xts = arena.tile([P, D], F32, tag="xts", bufs=2, name="xts")
                nc.sync.dma_start(xts[:], x_r[sc])
                n1s = arena.tile([P, D], BF, tag="n1s", bufs=2, name="n1s")
                _emit_ln_chunk(nc, small, xts[:], n1s[:], n1s[:], ln1a, ln1b)
                for cb in range(DC):
                    tpB = ps1.tile([P, P], BF, tag="tpB", bufs=4, name="tpB")
                    nc.tensor.transpose(tpB[:], n1s[:, ts(cb, P)], ident_b[:])
                    nc.vector.tensor_copy(n1T[:, cb, ts(sc, P)], tpB[:])

        # attention helpers ---------------------------------------------------
        def qproj_head(psQ, ps_tp8, hp):
            """q projection for feature chunk oc=hp -> qT8 + qh8e slices."""
            for b in range(2):
                pb = psQ.tile([P, 512], F32, tag="pb", bufs=1, name="pb")
                for kc in range(DC):
                    nc.tensor.matmul(
                        pb[:], wq_sb[:, kc, ts(hp, P)],
                        n1T[:, kc, ds(512 * b, 512)],
                        start=(kc == 0), stop=(kc == DC - 1),
                    )
                nc.vector.tensor_scalar(
                    qT8[:, hp, ds(512 * b, 512)], pb[:],
                    bq_sb[:, ds(hp, 1)], None, OP.add,
                )
            for sc in range(SC):
                tp8 = ps_tp8.tile([P, P, 2], FP8, tag="tp8", bufs=1,
                                  name="tp8")
                nc.tensor.transpose(
                    tp8[:, :, ds(0, 1)], qT8[:, hp, ts(sc, P)], ident8[:])
                for hl in range(2):
                    nc.vector.tensor_scalar_mul(
                        qh8e[:, sc, ds((2 * hp + hl) * HW, DK)],
                        tp8[:, ds(hl * DK, DK), 0], CTX_SCALE,
                    )

        def attn_head(psS, psC, hp, half):
            """scores+exp+ctx for head pair hp, query half `half`."""
            ec8 = arena.tile([P, SC, 1024], FP8, tag="ec8", bufs=2, name="ec8")
            for c in range(SC):
                sp = psS.tile([P, 1024], F32, tag="sp", bufs=2, name="sp")
                for hl in range(2):
                    lo = hl * DK
                    nc.tensor.matmul(
                        sp[:, ds(hl * 512, 512)],
                        qT8[ds(lo, DK), hp, ts(c, P)],
                        qT8[ds(lo, DK), hp, ds(512 * half, 512)],
                        start=True, stop=True,
                        tile_position=(lo, 0),
                    )
                nc.scalar.activation(
                    ec8[:, c], sp[:], AF.Exp, bias=ebias[:], scale=0.125,
                )
                if not mask_all_ones:
                    nc.vector.tensor_scalar_mul(
                        ec8[:, c], ec8[:, c], m01_sb[:, ds(c, 1)],
                    )
            for hl in range(2):
                h = 2 * hp + hl
                for qc in range(4):
                    cxp = psC.tile([P, 128], F32, tag="cxp", bufs=2,
                                   name="cxp")
                    for ci in range(SC // 2):
                        nc.tensor.matmul(
                            cxp[:, ds(0, HW)],
                            ec8[:, ds(2 * ci, 2),
                                ds(hl * 512 + qc * P, P)],
                            qh8e[:, ds(2 * ci, 2), ds(h * HW, HW)],
                            start=(ci == 0), stop=(ci == SC // 2 - 1),
                            perf_mode=DR,
                        )
                    rz = small.tile([P, 1], F32, tag="rz", bufs=3, name="rz")
                    nc.vector.reciprocal_approx_fast(rz[:], cxp[:, ds(DK, 1)])
                    nc.vector.tensor_scalar(
                        ctxN[:, half * 4 + qc, ds(h * DK, DK)],
                        cxp[:, ds(0, DK)], rz[:], None, OP.mult,
                    )

        def ctx_transpose_half(ps_tp, half):
            for sl in range(4):
                sc = half * 4 + sl
                for oc in range(DC):
                    tpC8 = ps_tp.tile([P, P, 2], FP8, tag="tpX", bufs=1,
                                      name="tpC8")
                    nc.tensor.transpose(
                        tpC8[:, :, ds(0, 1)], ctxN[:, sc, ts(oc, P)],
                        ident8[:])
                    nc.vector.tensor_copy(
                        ctxT8_t[:, oc, ts(sc, P)], tpC8[:, :, 0])

        def wo_half(psW, half):
            for sl in range(4):
                sc = half * 4 + sl
                xre = arena.tile([P, D], F32, tag="xre", bufs=1, name="xre")
                nc.sync.dma_start(xre[:], x_r[sc])
                for dh in range(2):
                    wp = psW.tile([P, 512], F32, tag="wp", bufs=1, name="wp")
                    for oi in range(DC // 2):
                        nc.tensor.matmul(
                            wp[:],
                            ctxT8_t[:, ds(2 * oi, 2), ts(sc, P)],
                            woT8_sb[:, ds(2 * oi, 2), ds(512 * dh, 512)],
                            start=(oi == 0), stop=(oi == DC // 2 - 1),
                            perf_mode=DR,
                        )
                    nc.vector.scalar_tensor_tensor(
                        res1[:, sc, ds(512 * dh, 512)], wp[:], OUT_SCALE,
                        xre[:, ds(512 * dh, 512)], OP.mult, OP.add,
                    )
                    nc.vector.tensor_add(
                        res1[:, sc, ds(512 * dh, 512)],
                        res1[:, sc, ds(512 * dh, 512)],
                        bo_rep[:, ds(512 * dh, 512)],
                    )

        def ln2_half(ps_tp, half, n2Th):
            for sl in range(4):
                sc = half * 4 + sl
                n2s = arena.tile([P, D], BF, tag="n2s", bufs=2, name="n2s")
                _emit_ln_chunk(nc, small, res1[:, sc], n2s[:], n2s[:],
                               ln2a, ln2b)
                for cb in range(DC):
                    tpC = ps_tp.tile([P, P], BF, tag="tpX", bufs=1,
                                     name="tpC")
                    nc.tensor.transpose(tpC[:], n2s[:, ts(cb, P)], ident_b[:])
                    nc.vector.tensor_copy(n2Th[:, cb, ts(sl, P)], tpC[:])
                nc.vector.tensor_add(res1[:, sc], res1[:, sc], b2_rep[:])

        def ffn1_chunk(psF, wsp, n2Th, h1t, lfc, fc, relu_on_scalar):
            wts = wsp.tile([P, DC, P], BF, tag="w1s", bufs=3, name="w1s")
            (nc.sync if fc % 2 == 0 else nc.gpsimd).dma_start(
                wts[:], w1L_d[:, fc])
            fp = psF.tile([P, 512], F32, tag="f1ps", bufs=2, name="f1ps")
            for dc in range(DC):
                nc.tensor.matmul(
                    fp[:], wts[:, dc], n2Th[:, dc, :],
                    start=(dc == 0), stop=(dc == DC - 1),
                )
            if relu_on_scalar:
                nc.scalar.activation(
                    h1t[:, lfc], fp[:], AF.Relu, bias=b1_sb[:, ds(fc, 1)],
                )
            else:
                nc.vector.tensor_scalar(
                    h1t[:, lfc], fp[:], b1_sb[:, ds(fc, 1)], 0.0,
                    OP.add, OP.max,
                )

        def ffn2_drain(half, dh, ops):
            for sl in range(4):
                sc = half * 4 + sl
                nc.vector.tensor_add(
                    res1[:, sc, ds(512 * dh, 512)], ops[sl][:],
                    res1[:, sc, ds(512 * dh, 512)],
                )
                (nc.gpsimd if sl % 2 == 0 else nc.scalar).dma_start(
                    out_r[sc][:, ds(512 * dh, 512)],
                    res1[:, sc, ds(512 * dh, 512)],
                )

        def ffn2_mms(ops, h1at, w2t, fc2):
            for fi in range(2):
                fc = 2 * fc2 + fi
                h1t, lfc = h1at(fc)
                for sl in range(4):
                    nc.tensor.matmul(
                        ops[sl][:], h1t[:, lfc, ts(sl, P)], w2t[:, fi],
                        start=(fc == 0), stop=(fc == FC - 1),
                    )

        # ================== phase 2: qproj pipeline + attention half 0 ======
        ctxT8_t = None
        with tc.tile_pool(name="wstream", bufs=1) as wsp:
            with tc.tile_pool(name="psSp", bufs=1, space="PSUM") as psS, \
                 tc.tile_pool(name="psCx", bufs=1, space="PSUM") as psC:
                with tc.tile_pool(name="psQ", bufs=1, space="PSUM") as psQ:
                    for hp in range(DC):
                        qproj_head(psQ, psQ, hp)
                        if hp >= 1:
                            attn_head(psS, psC, hp - 1, 0)
                    attn_head(psS, psC, DC - 1, 0)

                # ========= phase 3: ctxT8(0), wo(0), ln2(0) =================
                ctxT8_t = arena.tile([P, DC, S], FP8, tag="n1T_ctxT8",
                                     name="ctxT8")
                n2Th0 = arena.tile([P, DC, 512], BF, tag="wq_n2t",
                                   bufs=2, name="n2Th0")
                with tc.tile_pool(name="ps3", bufs=1, space="PSUM") as ps3:
                    ctx_transpose_half(ps3, 0)
                    wo_half(ps3, 0)
                    ln2_half(ps3, 0, n2Th0)

                # ==== phase 4: attention half 1 with ffn1(0) interleaved ====
                h1a = arena.tile([P, FC, 512], BF, tag="xt_h1", name="h1a")
                with tc.tile_pool(name="ps4", bufs=1, space="PSUM") as ps4:
                    for hp in range(DC):
                        attn_head(psS, psC, hp, 1)
                        for fc in range(4 * hp, 4 * hp + 4):
                            ffn1_chunk(ps4, wsp, n2Th0, h1a, fc, fc, False)

            # =========== phase 5: ctxT8(1), wo(1), ln2(1) ===================
            n2Th1 = arena.tile([P, DC, 512], BF, tag="wq_n2t", bufs=2,
                               name="n2Th1")
            with tc.tile_pool(name="ps5", bufs=1, space="PSUM") as ps5:
                ctx_transpose_half(ps5, 1)
                wo_half(ps5, 1)
                ln2_half(ps5, 1, n2Th1)

            # ====== phases 6-7: interleaved FFN tail (passes A, B, C) =======
            # h1(half 1) lands in SBUF regions dead after attention
            h1b_parts = [
                arena.tile([P, 8, 512], BF, tag="ec8", bufs=2, name="h1b0"),
                arena.tile([P, 8, 512], BF, tag="ec8", bufs=2, name="h1b1"),
                arena.tile([P, 8, 512], BF, tag="qT8", name="h1b2"),
                arena.tile([P, 8, 512], BF, tag="qh8e", name="h1b3"),
            ]
            h1a_at = lambda fc: (h1a, fc)
            h1b_at = lambda fc: (h1b_parts[fc // 8], fc % 8)

            # pass A: ffn2(half0, dh0) + ffn1(half1) interleaved
            with tc.tile_pool(name="psA6", bufs=1, space="PSUM") as psA6:
                opsA = [psA6.tile([P, 512], F32, tag="f2psA", bufs=4,
                                  name="f2psA") for _ in range(4)]
                for fc2 in range(FC // 2):
                    w2t = wsp.tile([P, 2, 512], BF, tag="w2s", bufs=3,
                                   name="w2s")
                    (nc.sync if fc2 % 2 == 0 else nc.scalar).dma_start(
                        w2t[:], w2L_d[:, 0, fc2])
                    ffn2_mms(opsA, h1a_at, w2t, fc2)
                    for fc in (2 * fc2, 2 * fc2 + 1):
                        h1t, lfc = h1b_at(fc)
                        ffn1_chunk(psA6, wsp, n2Th1, h1t, lfc, fc, True)
                ffn2_drain(0, 0, opsA)

            # pass B: ffn2(half0, dh1) + ffn2(half1, dh1), shared w2 stream
            with tc.tile_pool(name="psB6", bufs=1, space="PSUM") as psB6:
                opsB0 = [psB6.tile([P, 512], F32, tag="f2psB0", bufs=4,
                                   name="f2psB0") for _ in range(4)]
                opsB1 = [psB6.tile([P, 512], F32, tag="f2psB1", bufs=4,
                                   name="f2psB1") for _ in range(4)]
                for fc2 in range(FC // 2):
                    w2t = wsp.tile([P, 2, 512], BF, tag="w2s", bufs=3,
                                   name="w2s")
                    (nc.sync if fc2 % 2 == 0 else nc.scalar).dma_start(
                        w2t[:], w2L_d[:, 1, fc2])
                    ffn2_mms(opsB0, h1a_at, w2t, fc2)
                    ffn2_mms(opsB1, h1b_at, w2t, fc2)
                ffn2_drain(0, 1, opsB0)
                ffn2_drain(1, 1, opsB1)

            # pass C: ffn2(half1, dh0)
            with tc.tile_pool(name="psC6", bufs=1, space="PSUM") as psC6:
                opsC = [psC6.tile([P, 512], F32, tag="f2psC", bufs=4,
                                  name="f2psC") for _ in range(4)]
                for fc2 in range(FC // 2):
                    w2t = wsp.tile([P, 2, 512], BF, tag="w2s", bufs=3,
                                   name="w2s")
                    (nc.sync if fc2 % 2 == 0 else nc.scalar).dma_start(
                        w2t[:], w2L_d[:, 0, fc2])
                    ffn2_mms(opsC, h1b_at, w2t, fc2)
                ffn2_drain(1, 0, opsC)

    nc.compile()
    return nc


def _prep_inputs(inputs):
    f32 = lambda a: np.ascontiguousarray(np.asarray(a, dtype=np.float32))
    bfT = lambda a: np.ascontiguousarray(
        np.asarray(a, dtype=np.float32).T.astype(ml_dtypes.bfloat16))
    x = f32(inputs["x"])                      # [B, S, D]
    mask = np.asarray(inputs["src_mask"])     # [B, 1, 1, S] int32
    wqT = bfT(inputs["wq"])                   # [D, D] (in, out)
    woT8 = np.ascontiguousarray(
        (np.asarray(inputs["wo"], dtype=np.float32).T * WO_SCALE)
        .astype(ml_dtypes.float8_e4m3))
    w1 = np.asarray(inputs["w1"], dtype=np.float32)      # [DFF, D]
    w2 = np.asarray(inputs["w2"], dtype=np.float32)      # [D, DFF]
    # w1L[p, fc, dc, f] = w1[fc*128+f, dc*128+p]; 2KB-contiguous DMA chunks
    w1L = np.ascontiguousarray(
        w1.reshape(FC, P, DC, P).transpose(3, 0, 2, 1)
        .astype(ml_dtypes.bfloat16))
    # w2L[p, dh, fc2, i, d] = w2[dh*512+d, (2*fc2+i)*128+p]
    w2L = np.ascontiguousarray(
        w2.reshape(2, 512, FC // 2, 2, P).transpose(4, 0, 2, 3, 1)
        .astype(ml_dtypes.bfloat16))
    bq_v = np.ascontiguousarray(f32(inputs["bq"]).reshape(DC, P).T)
    b1_v = np.ascontiguousarray(f32(inputs["b1"]).reshape(FC, P).T)
    bo_rep = np.ascontiguousarray(np.tile(f32(inputs["bo"]), (P, 1)))
    b2_rep = np.ascontiguousarray(np.tile(f32(inputs["b2"]), (P, 1)))
    scal = lambda k: float(np.asarray(inputs[k]).reshape(-1)[0])
    ln = (scal("ln1_a"), scal("ln1_b"), scal("ln2_a"), scal("ln2_b"))
    mask_all_ones = bool((mask != 0).all())

    shared = dict(wqT=wqT, woT8=woT8, w1L=w1L, w2L=w2L, bq_v=bq_v, b1_v=b1_v,
                  bo_rep=bo_rep, b2_rep=b2_rep)
    in_maps = []
    for b in range(NB):
        m = dict(shared)
        m["x"] = np.ascontiguousarray(x[b])
        if not mask_all_ones:
            m01 = (mask[b].reshape(S) != 0).astype(np.float32)
            m["m01_v"] = np.ascontiguousarray(m01.reshape(SC, P).T)
        in_maps.append(m)
    return in_maps, ln, mask_all_ones


last_nc = None
last_in_maps = None


def kernel(**inputs):
    global last_nc, last_in_maps
    in_maps, ln, mask_all_ones = _prep_inputs(inputs)
    nc = build_program(*ln, mask_all_ones)
    last_nc, last_in_maps = nc, in_maps
    res = bass_utils.run_bass_kernel_spmd(
        nc, in_maps, core_ids=list(range(NB)), trace=False,
    )
    out = np.stack([np.asarray(res.results[b]["out"]) for b in range(NB)])
    return out.astype(np.float32)
